# revision 35
# baseline (speedup 1.0000x reference)
"""AttMemoryLayer Trainium2 kernel (8 NeuronCores, batch-parallel).

Math (per batch b):
    scores[s] = sum_d memory[b,s,d] * W[:D]  (+ c_b, c_b = aspect[b]@W[D:] + b)
    p = exp(tanh(scores))          # tanh in [-1,1] => no max-subtraction needed
    out[b] = (sum_s p[s] * memory[b,s,:]) / sum_s p[s]

Distribution: data-parallel over B=64 across 8 cores (8 batches/core),
W/b replicated.  No collectives.  The tiny W/aspect preprocessing (Wm
broadcast, Wa columns, aspect transpose) is plain numpy layout prep on the
host; all O(B*S*D) work runs on-device.

Per-core dataflow (per batch: 2 chunks of 16 s-slices):
  - SWDGE cast-DMA streams each 2MB chunk f32->bf16 as [128, 16, 256]
    (partition = s_outer, free = (s_inner, d)); DMA is the wall-clock floor
    (32MB f32 HBM read/core).
  - scores (the free-dim contraction PE cannot do from this layout) is split
    across both free engines:
      * slices [0:11): one VectorE bf16 tensor_tensor multiply (2x mode,
        broadcast Wm), a bf16 binary tree of in-place halving adds, then one
        f32-accumulating reduce_sum of [.,.,32];
      * slices [11:16): multiply into prodB, then ScalarE
        activation(Copy, accum_out) per slice (dump target in PSUM).
  - ScalarE: tanh (bias = per-batch c, broadcast via a K=1 ones matmul), exp.
  - PE: 16 accumulating bf16 matmuls per chunk, lhsT = p[:,j] (stationary,
    1 column), rhs = memory slice streams at 1 col/cycle.
  - Per-batch normalizer: ones-matmul partition reduce -> reciprocal ->
    ScalarE Copy-with-scale folds the divide into the PSUM->stage copy;
    one 8KB output DMA at the end.

Measured on the axon-tunneled TRN2: ~108-111us NEFF exec, rel err ~2.9e-3
vs the f32 reference (bf16 memory quantization dominates the error).
"""
import sys

for _p in ("/opt/trn_rl_repo",):
    if _p not in sys.path:
        sys.path.append(_p)

import numpy as np

import concourse.bass as bass
import concourse.mybir as mybir
from concourse.tile import TileContext
from concourse.vector_clock import ScopedClock
from concourse.bass_utils import run_bass_kernel_spmd

F32 = mybir.dt.float32
BF16 = mybir.dt.bfloat16

B, S, D = 64, 4096, 256
NCORES = 8
BPC = B // NCORES          # batches per core
SO = 128                   # s_outer (partitions)
SI = S // SO               # s_inner per batch (32)
NCH = 2                    # compute chunks per batch (halves of one DMA)
CJ = SI // NCH             # s-slices per compute chunk (16)


def _split_multi_waits(nc, max_waits=1):
    """This container's walrus build rejects instructions carrying more than
    one sync-wait ("Too many sync wait commands").  Move extra waits onto
    single-wait NoOps inserted immediately before the instruction on the same
    engine; per-engine program order makes this semantics-preserving."""
    cnt = 0
    for bb in nc.main_func.blocks:
        newlist = []
        dirty = False
        for ins in bb.instructions:
            si = ins.sync_info
            if si is not None and si.on_wait and len(si.on_wait) > max_waits:
                waits = list(si.on_wait)
                head, tail = waits[:-max_waits], waits[-max_waits:]
                for w in head:
                    cnt += 1
                    newlist.append(
                        mybir.InstNoOp(
                            name=f"WSPLIT-{cnt}",
                            engine=ins.engine,
                            bass_nofuse=True,
                            sync_info=mybir.SyncInfo(on_wait=[w], on_update=[]),
                        )
                    )
                ins.sync_info = mybir.SyncInfo(
                    on_wait=tail, on_update=list(si.on_update or [])
                )
                dirty = True
            newlist.append(ins)
        if dirty:
            bb.instructions = newlist
    return cnt


class _TC(TileContext):
    """TileContext with a slimmer kernel tail: the drain still waits on all
    outstanding work (output visibility) and semaphores are still cleared
    (repeat-execution safety), but the second all-engine barrier is dropped
    -- NEFF completion already requires every engine stream (including the
    clears) to retire, so nothing can observe a stale semaphore."""

    def _drain_and_barrier(self, tick_clock, wait_clock):
        drain_inst = self.nc.sync.drain()
        wait_clock.add_sem_waits(
            drain_inst.ins, ScopedClock({None: tick_clock.global_clock})
        )
        self.nc.all_engine_barrier()
        popped = self.nc._tile_sem_poison_stack.pop()
        assert popped is self._sem_poison
        self.nc.clear_and_free_semaphores(list(self.sems.allocated().values()))


def build_nc():
    nc = bass.Bass(trn_type="TRN2")

    MEM = nc.dram_tensor("mem", [BPC, S, D], F32, kind="ExternalInput")
    WMB = nc.dram_tensor("wmb", [128, D], F32, kind="ExternalInput")
    ASPT = nc.dram_tensor("aspt", [128, 2, BPC], F32, kind="ExternalInput")
    WAC = nc.dram_tensor("wac", [128, 2], F32, kind="ExternalInput")
    BSC = nc.dram_tensor("bsc", [1, 1], F32, kind="ExternalInput")
    ONEC = nc.dram_tensor("onec", [128, 1], F32, kind="ExternalInput")
    ONER = nc.dram_tensor("oner", [1, 128], F32, kind="ExternalInput")
    WINV = nc.dram_tensor("winv", [1, D], F32, kind="ExternalInput")
    OUT = nc.dram_tensor("out", [1, BPC * D], F32, kind="ExternalOutput")

    mult = mybir.AluOpType.mult
    Act = mybir.ActivationFunctionType

    with _TC(nc) as tc:
        with (
            tc.tile_pool(name="const", bufs=1) as cpool,
            tc.tile_pool(name="mem", bufs=5) as mpool,
            tc.tile_pool(name="small", bufs=4) as spool,
            tc.tile_pool(name="prods", bufs=6) as prpool,
            tc.tile_pool(name="psums", bufs=1, space="PSUM") as pps,
            tc.tile_pool(name="psumm", bufs=2, space="PSUM") as ppm,
        ):
            # ---- constants / setup -------------------------------------
            wmb16 = cpool.tile([128, D], BF16)
            nc.gpsimd.dma_start(wmb16[:], WMB[:])  # f32 -> bf16 cast DMA
            aspt = cpool.tile([128, 2, BPC], F32)
            nc.sync.dma_start(aspt[:], ASPT[:])
            wac = cpool.tile([128, 2], F32)
            nc.sync.dma_start(wac[:], WAC[:])
            bsc = cpool.tile([1, 1], F32)
            nc.sync.dma_start(bsc[:], BSC[:])
            onec = cpool.tile([128, 1], F32)
            nc.sync.dma_start(onec[:], ONEC[:])
            oner = cpool.tile([1, 128], F32)
            nc.sync.dma_start(oner[:], ONER[:])
            winv = cpool.tile([1, D], F32)
            nc.sync.dma_start(winv[:], WINV[:])

            # c_row[1, BPC] = aspect @ Wa + b   (contract d over partitions)
            ps_c = pps.tile([1, BPC], F32)
            nc.tensor.matmul(ps_c[:], lhsT=wac[:, 0:1], rhs=aspt[:, 0, :],
                             start=True, stop=False)
            nc.tensor.matmul(ps_c[:], lhsT=wac[:, 1:2], rhs=aspt[:, 1, :],
                             start=False, stop=True)
            crow = cpool.tile([1, BPC], F32)
            nc.vector.tensor_scalar_add(crow[:], ps_c[:], bsc[0:1, 0:1])

            # c broadcast to all 128 partitions: ones_row.T @ c_row (K=1)
            ps_cb = pps.tile([128, BPC], F32)
            nc.tensor.matmul(ps_cb[:], lhsT=oner[:], rhs=crow[:],
                             start=True, stop=True)
            cb = cpool.tile([128, BPC], F32)
            nc.scalar.copy(cb[:], ps_cb[:])

            MAXCH = 4
            lpart = cpool.tile([128, BPC * MAXCH], F32)  # per-(batch,chunk) l
            stage = cpool.tile([1, BPC, D], F32)         # un-normalized outputs

            # ---- main loop: batches x chunks ---------------------------
            # Batch 0 uses four 1MB chunks so compute starts as early as
            # possible; later batches use two 2MB chunks (lowest DMA cost).
            plans = [[(16, 5)] * 2] * BPC
            for i in range(BPC):
                plan = plans[i]
                nch = len(plan)
                out_ps = ppm.tile([1, D], F32, tag="out_ps")
                s0 = 0
                for c, (cj, nsp) in enumerate(plan):
                    bts = mpool.tile([128, cj, D], BF16, tag="bt")
                    nc.gpsimd.dma_start(
                        bts[:],
                        MEM[i].rearrange("(so si) d -> so si d", so=SO)[
                            :, s0 : s0 + cj, :
                        ],
                    )
                    s0 += cj
                    # scores[s,j] = sum_d bts[s,j,d] * Wm[d]; one bf16 2x
                    # multiply, then reduction split across engines: slices
                    # [0:na) tree-reduce on VectorE, slices [na:cj) accum-
                    # reduce on ScalarE.
                    na = cj - nsp
                    prod = prpool.tile([128, cj, D], BF16, tag="prod")
                    nc.vector.tensor_tensor(
                        prod[:], bts[:],
                        wmb16[:, None, :].to_broadcast((128, cj, D)), mult,
                    )
                    scores = spool.tile([128, cj], F32, tag="scores")
                    dump = ppm.tile([128, D], F32, tag="dump")
                    for j in range(na, cj):
                        nc.scalar.activation(
                            dump[:], prod[:, j, :], Act.Copy,
                            accum_out=scores[:, j : j + 1],
                        )
                    tree = prpool.tile([128, na, 128], BF16, tag="tree")
                    nc.vector.tensor_add(tree[:], prod[:, 0:na, 0:128],
                                         prod[:, 0:na, 128:256])
                    nc.vector.tensor_add(tree[:, :, 0:64], tree[:, :, 0:64],
                                         tree[:, :, 64:128])
                    nc.vector.tensor_add(tree[:, :, 0:32], tree[:, :, 0:32],
                                         tree[:, :, 32:64])
                    nc.vector.tensor_add(tree[:, :, 0:16], tree[:, :, 0:16],
                                         tree[:, :, 16:32])
                    nc.vector.reduce_sum(scores[:, 0:na], tree[:, :, 0:16],
                                         axis=mybir.AxisListType.X)

                    th = spool.tile([128, cj], F32, tag="th")
                    nc.scalar.activation(th[:], scores[:], Act.Tanh,
                                         bias=cb[:, i : i + 1])
                    p16 = spool.tile([128, cj], BF16, tag="p16")
                    nc.scalar.activation(p16[:], th[:], Act.Exp)
                    nc.vector.reduce_sum(
                        lpart[:, i * MAXCH + c : i * MAXCH + c + 1], p16[:],
                        axis=mybir.AxisListType.X,
                    )

                    for j in range(cj):
                        nc.tensor.matmul(
                            out_ps[:], lhsT=p16[:, j : j + 1],
                            rhs=prod[:, j, :],
                            start=(c == 0 and j == 0),
                            stop=(c == nch - 1 and j == cj - 1),
                        )
                # per-batch normalizer: partition-reduce l, reciprocal,
                # and scale the PSUM row on its way into the staging tile
                ps_l = ppm.tile([1, MAXCH], F32, tag="ps_l")
                nc.tensor.matmul(ps_l[:, 0:nch], lhsT=onec[:],
                                 rhs=lpart[:, i * MAXCH : i * MAXCH + nch],
                                 start=True, stop=True)
                lsum = spool.tile([1, 1], F32, tag="lsum")
                nc.vector.reduce_sum(lsum[:], ps_l[:, 0:nch],
                                     axis=mybir.AxisListType.X)
                lrec = spool.tile([1, 1], F32, tag="lrec")
                nc.vector.reciprocal(lrec[:], lsum[:])
                nc.scalar.activation(stage[0:1, i, :], out_ps[:], Act.Copy,
                                     bias=0.0, scale=lrec[0:1, 0:1])
                nc.vector.tensor_tensor(stage[0:1, i, :], stage[0:1, i, :],
                                        winv[0:1, :], mult)

            nc.sync.dma_start(OUT[:], stage[:].rearrange("p i j -> p (i j)"))

    _split_multi_waits(nc)
    return nc


_NC_CACHE = None


def _get_nc():
    global _NC_CACHE
    if _NC_CACHE is None:
        _NC_CACHE = build_nc()
    return _NC_CACHE


def make_in_maps(aspect, memory, W, b):
    aspect = np.asarray(aspect, dtype=np.float32).reshape(B, D)
    memory = np.ascontiguousarray(np.asarray(memory, dtype=np.float32))
    W = np.asarray(W, dtype=np.float32).reshape(2 * D)
    b = np.asarray(b, dtype=np.float32).reshape(1)

    wmb = np.ascontiguousarray(np.tile(W[:D][None, :], (128, 1)))
    wm16 = wmb[0].astype(np.dtype("uint16").newbyteorder("="), copy=False)  # placeholder
    wmq = W[:D].astype(np.float32)
    # quantize Wm exactly as the cast-DMA does (bf16 round-to-nearest-even)
    import ml_dtypes
    wmq16 = wmq.astype(ml_dtypes.bfloat16).astype(np.float32)
    winv = np.where(wmq16 == 0.0, 0.0, 1.0 / wmq16).astype(np.float32)[None, :]
    wac = np.ascontiguousarray(W[D:].reshape(2, 128).T)
    bsc = b.reshape(1, 1)
    onec = np.ones((128, 1), dtype=np.float32)
    oner = np.ones((1, 128), dtype=np.float32)

    in_maps = []
    for c in range(NCORES):
        asp = aspect[c * BPC : (c + 1) * BPC]          # [BPC, D]
        aspt = np.ascontiguousarray(
            asp.T.reshape(2, 128, BPC).transpose(1, 0, 2)
        )                                               # [128, 2, BPC]
        in_maps.append(
            {
                "mem": memory[c * BPC : (c + 1) * BPC],
                "wmb": wmb,
                "aspt": aspt,
                "wac": wac,
                "bsc": bsc,
                "onec": onec,
                "oner": oner,
                "winv": winv,
            }
        )
    return in_maps


def run(inputs, trace=False):
    """Returns (out [B, D] float32, exec_time_ns or None)."""
    nc = _get_nc()
    in_maps = make_in_maps(**inputs)
    res = run_bass_kernel_spmd(
        nc, in_maps, core_ids=list(range(NCORES)), trace=trace
    )
    out = np.concatenate(
        [res.results[c]["out"].reshape(BPC, D) for c in range(NCORES)], axis=0
    )
    return out, res.exec_time_ns


def kernel(aspect, memory, W, b):
    out, _ = run(dict(aspect=aspect, memory=memory, W=W, b=b))
    return out


# revision 36
# speedup vs baseline: 1.0353x; 1.0353x over previous
"""AttMemoryLayer Trainium2 kernel (8 NeuronCores, batch-parallel).

Math (per batch b):
    scores[s] = sum_d memory[b,s,d] * W[:D]  (+ c_b, c_b = aspect[b]@W[D:] + b)
    p = exp(tanh(scores))          # tanh in [-1,1] => no max-subtraction needed
    out[b] = (sum_s p[s] * memory[b,s,:]) / sum_s p[s]

Distribution: data-parallel over B=64 across 8 cores (8 batches/core), W/b
replicated, no collectives.  Tiny W/aspect preprocessing (Wm broadcast, Wa
columns, aspect transpose, 1/Wm) is host-side numpy layout prep; all
O(B*S*D) work runs on-device.

Per-core dataflow (per batch: 2 chunks of 16 s-slices):
  - SWDGE cast-DMA streams each 2MB chunk f32->bf16 as [128, 16, 256]
    (partition = s_outer, free = (s_inner, d)); the 32MB f32 HBM read/core
    is the wall-clock floor.
  - One VectorE bf16 2x multiply per chunk: prod = chunk * Wm (broadcast).
  - scores (a free-dim contraction PE cannot do in this layout) split across
    engines: 11 slices tree-reduce on VectorE (4 bf16 halving adds into a
    separate tile + one f32-accumulating reduce_sum over [.,.,16]); 5 slices
    accum-reduce on ScalarE via activation(Copy, accum_out), dump in PSUM.
  - ScalarE: tanh (bias = per-batch c, broadcast via a K=1 ones matmul), exp.
  - PE: 16 accumulating bf16 matmuls per chunk read PROD (not raw memory),
    lhsT = p[:,j] stationary 1-column; the Wm factor is undone at the end by
    one per-batch row multiply with host-precomputed 1/Wm.  Pooling from prod
    keeps memory-tile lifetimes to just the multiply, so DMA slot recycling
    never waits on the exp->matmul chain (this removed both DMA stalls and
    most run-to-run variance).
  - Per-batch normalizer: ones-matmul partition reduce -> reciprocal ->
    ScalarE Copy-with-scale folds the divide into the PSUM->stage copy; one
    8KB output DMA at the end.  Custom slim TileContext tail (one barrier).

Measured on the axon-tunneled TRN2: ~108-110us NEFF exec in quiet fleet
phases (116-129us when the shared tunnel degrades), rel err ~3e-3 vs the
f32 reference (bf16 memory quantization dominates the error).
"""
import sys

for _p in ("/opt/trn_rl_repo",):
    if _p not in sys.path:
        sys.path.append(_p)

import numpy as np

import concourse.bass as bass
import concourse.mybir as mybir
from concourse.tile import TileContext
from concourse.vector_clock import ScopedClock
from concourse.bass_utils import run_bass_kernel_spmd

F32 = mybir.dt.float32
BF16 = mybir.dt.bfloat16

B, S, D = 64, 4096, 256
NCORES = 8
BPC = B // NCORES          # batches per core
SO = 128                   # s_outer (partitions)
SI = S // SO               # s_inner per batch (32)
NCH = 2                    # compute chunks per batch (halves of one DMA)
CJ = SI // NCH             # s-slices per compute chunk (16)


def _split_multi_waits(nc, max_waits=1):
    """This container's walrus build rejects instructions carrying more than
    one sync-wait ("Too many sync wait commands").  Move extra waits onto
    single-wait NoOps inserted immediately before the instruction on the same
    engine; per-engine program order makes this semantics-preserving."""
    cnt = 0
    for bb in nc.main_func.blocks:
        newlist = []
        dirty = False
        for ins in bb.instructions:
            si = ins.sync_info
            if si is not None and si.on_wait and len(si.on_wait) > max_waits:
                waits = list(si.on_wait)
                head, tail = waits[:-max_waits], waits[-max_waits:]
                for w in head:
                    cnt += 1
                    newlist.append(
                        mybir.InstNoOp(
                            name=f"WSPLIT-{cnt}",
                            engine=ins.engine,
                            bass_nofuse=True,
                            sync_info=mybir.SyncInfo(on_wait=[w], on_update=[]),
                        )
                    )
                ins.sync_info = mybir.SyncInfo(
                    on_wait=tail, on_update=list(si.on_update or [])
                )
                dirty = True
            newlist.append(ins)
        if dirty:
            bb.instructions = newlist
    return cnt


class _TC(TileContext):
    """TileContext with a slimmer kernel tail: the drain still waits on all
    outstanding work (output visibility) and semaphores are still cleared
    (repeat-execution safety), but the second all-engine barrier is dropped
    -- NEFF completion already requires every engine stream (including the
    clears) to retire, so nothing can observe a stale semaphore."""

    def _drain_and_barrier(self, tick_clock, wait_clock):
        drain_inst = self.nc.sync.drain()
        wait_clock.add_sem_waits(
            drain_inst.ins, ScopedClock({None: tick_clock.global_clock})
        )
        self.nc.all_engine_barrier()
        popped = self.nc._tile_sem_poison_stack.pop()
        assert popped is self._sem_poison
        self.nc.clear_and_free_semaphores(list(self.sems.allocated().values()))


def build_nc():
    nc = bass.Bass(trn_type="TRN2")

    MEM = nc.dram_tensor("mem", [BPC, S, D], F32, kind="ExternalInput")
    WMB = nc.dram_tensor("wmb", [128, D], F32, kind="ExternalInput")
    ASPT = nc.dram_tensor("aspt", [128, 2, BPC], F32, kind="ExternalInput")
    WAC = nc.dram_tensor("wac", [128, 2], F32, kind="ExternalInput")
    BSC = nc.dram_tensor("bsc", [1, 1], F32, kind="ExternalInput")
    ONEC = nc.dram_tensor("onec", [128, 1], F32, kind="ExternalInput")
    ONER = nc.dram_tensor("oner", [1, 128], F32, kind="ExternalInput")
    WINV = nc.dram_tensor("winv", [1, D], F32, kind="ExternalInput")
    OUT = nc.dram_tensor("out", [1, BPC * D], F32, kind="ExternalOutput")

    mult = mybir.AluOpType.mult
    Act = mybir.ActivationFunctionType

    with _TC(nc) as tc:
        with (
            tc.tile_pool(name="const", bufs=1) as cpool,
            tc.tile_pool(name="mem", bufs=5) as mpool,
            tc.tile_pool(name="small", bufs=4) as spool,
            tc.tile_pool(name="prods", bufs=6) as prpool,
            tc.tile_pool(name="psums", bufs=1, space="PSUM") as pps,
            tc.tile_pool(name="psumm", bufs=2, space="PSUM") as ppm,
        ):
            # ---- constants / setup -------------------------------------
            wmb16 = cpool.tile([128, D], BF16)
            nc.gpsimd.dma_start(wmb16[:], WMB[:])  # f32 -> bf16 cast DMA
            aspt = cpool.tile([128, 2, BPC], F32)
            nc.sync.dma_start(aspt[:], ASPT[:])
            wac = cpool.tile([128, 2], F32)
            nc.sync.dma_start(wac[:], WAC[:])
            bsc = cpool.tile([1, 1], F32)
            nc.sync.dma_start(bsc[:], BSC[:])
            onec = cpool.tile([128, 1], F32)
            nc.sync.dma_start(onec[:], ONEC[:])
            oner = cpool.tile([1, 128], F32)
            nc.sync.dma_start(oner[:], ONER[:])
            winv = cpool.tile([1, D], F32)
            nc.sync.dma_start(winv[:], WINV[:])

            # c_row[1, BPC] = aspect @ Wa + b   (contract d over partitions)
            ps_c = pps.tile([1, BPC], F32)
            nc.tensor.matmul(ps_c[:], lhsT=wac[:, 0:1], rhs=aspt[:, 0, :],
                             start=True, stop=False)
            nc.tensor.matmul(ps_c[:], lhsT=wac[:, 1:2], rhs=aspt[:, 1, :],
                             start=False, stop=True)
            crow = cpool.tile([1, BPC], F32)
            nc.vector.tensor_scalar_add(crow[:], ps_c[:], bsc[0:1, 0:1])

            # c broadcast to all 128 partitions: ones_row.T @ c_row (K=1)
            ps_cb = pps.tile([128, BPC], F32)
            nc.tensor.matmul(ps_cb[:], lhsT=oner[:], rhs=crow[:],
                             start=True, stop=True)
            cb = cpool.tile([128, BPC], F32)
            nc.scalar.copy(cb[:], ps_cb[:])

            MAXCH = 4
            lpart = cpool.tile([128, BPC * MAXCH], F32)  # per-(batch,chunk) l
            stage = cpool.tile([1, BPC, D], F32)         # un-normalized outputs

            # ---- main loop: batches x chunks ---------------------------
            # Uniform two 2MB chunks per batch, 5 ScalarE-reduced slices per
            # chunk: every non-uniform variation of this plan measured worse.
            plans = [[(16, 5)] * 2] * BPC
            for i in range(BPC):
                plan = plans[i]
                nch = len(plan)
                out_ps = ppm.tile([1, D], F32, tag="out_ps")
                s0 = 0
                for c, (cj, nsp) in enumerate(plan):
                    bts = mpool.tile([128, cj, D], BF16, tag="bt")
                    nc.gpsimd.dma_start(
                        bts[:],
                        MEM[i].rearrange("(so si) d -> so si d", so=SO)[
                            :, s0 : s0 + cj, :
                        ],
                    )
                    s0 += cj
                    # scores[s,j] = sum_d bts[s,j,d] * Wm[d]; one bf16 2x
                    # multiply, then reduction split across engines: slices
                    # [0:na) tree-reduce on VectorE, slices [na:cj) accum-
                    # reduce on ScalarE.
                    na = cj - nsp
                    prod = prpool.tile([128, cj, D], BF16, tag="prod")
                    nc.vector.tensor_tensor(
                        prod[:], bts[:],
                        wmb16[:, None, :].to_broadcast((128, cj, D)), mult,
                    )
                    scores = spool.tile([128, cj], F32, tag="scores")
                    dump = ppm.tile([128, D], F32, tag="dump")
                    for j in range(na, cj):
                        nc.scalar.activation(
                            dump[:], prod[:, j, :], Act.Copy,
                            accum_out=scores[:, j : j + 1],
                        )
                    tree = prpool.tile([128, na, 128], BF16, tag="tree")
                    nc.vector.tensor_add(tree[:], prod[:, 0:na, 0:128],
                                         prod[:, 0:na, 128:256])
                    nc.vector.tensor_add(tree[:, :, 0:64], tree[:, :, 0:64],
                                         tree[:, :, 64:128])
                    nc.vector.tensor_add(tree[:, :, 0:32], tree[:, :, 0:32],
                                         tree[:, :, 32:64])
                    nc.vector.tensor_add(tree[:, :, 0:16], tree[:, :, 0:16],
                                         tree[:, :, 16:32])
                    nc.vector.reduce_sum(scores[:, 0:na], tree[:, :, 0:16],
                                         axis=mybir.AxisListType.X)

                    th = spool.tile([128, cj], F32, tag="th")
                    nc.scalar.activation(th[:], scores[:], Act.Tanh,
                                         bias=cb[:, i : i + 1])
                    p16 = spool.tile([128, cj], BF16, tag="p16")
                    nc.scalar.activation(p16[:], th[:], Act.Exp)
                    nc.vector.reduce_sum(
                        lpart[:, i * MAXCH + c : i * MAXCH + c + 1], p16[:],
                        axis=mybir.AxisListType.X,
                    )

                    for j in range(cj):
                        nc.tensor.matmul(
                            out_ps[:], lhsT=p16[:, j : j + 1],
                            rhs=prod[:, j, :],
                            start=(c == 0 and j == 0),
                            stop=(c == nch - 1 and j == cj - 1),
                        )
                # per-batch normalizer: partition-reduce l, reciprocal,
                # and scale the PSUM row on its way into the staging tile
                ps_l = ppm.tile([1, MAXCH], F32, tag="ps_l")
                nc.tensor.matmul(ps_l[:, 0:nch], lhsT=onec[:],
                                 rhs=lpart[:, i * MAXCH : i * MAXCH + nch],
                                 start=True, stop=True)
                lsum = spool.tile([1, 1], F32, tag="lsum")
                nc.vector.reduce_sum(lsum[:], ps_l[:, 0:nch],
                                     axis=mybir.AxisListType.X)
                lrec = spool.tile([1, 1], F32, tag="lrec")
                nc.vector.reciprocal(lrec[:], lsum[:])
                nc.scalar.activation(stage[0:1, i, :], out_ps[:], Act.Copy,
                                     bias=0.0, scale=lrec[0:1, 0:1])
                nc.vector.tensor_tensor(stage[0:1, i, :], stage[0:1, i, :],
                                        winv[0:1, :], mult)

            nc.sync.dma_start(OUT[:], stage[:].rearrange("p i j -> p (i j)"))

    _split_multi_waits(nc)
    return nc


_NC_CACHE = None


def _get_nc():
    global _NC_CACHE
    if _NC_CACHE is None:
        _NC_CACHE = build_nc()
    return _NC_CACHE


def make_in_maps(aspect, memory, W, b):
    aspect = np.asarray(aspect, dtype=np.float32).reshape(B, D)
    memory = np.ascontiguousarray(np.asarray(memory, dtype=np.float32))
    W = np.asarray(W, dtype=np.float32).reshape(2 * D)
    b = np.asarray(b, dtype=np.float32).reshape(1)

    wmb = np.ascontiguousarray(np.tile(W[:D][None, :], (128, 1)))
    wm16 = wmb[0].astype(np.dtype("uint16").newbyteorder("="), copy=False)  # placeholder
    wmq = W[:D].astype(np.float32)
    # quantize Wm exactly as the cast-DMA does (bf16 round-to-nearest-even)
    import ml_dtypes
    wmq16 = wmq.astype(ml_dtypes.bfloat16).astype(np.float32)
    winv = np.where(wmq16 == 0.0, 0.0, 1.0 / wmq16).astype(np.float32)[None, :]
    wac = np.ascontiguousarray(W[D:].reshape(2, 128).T)
    bsc = b.reshape(1, 1)
    onec = np.ones((128, 1), dtype=np.float32)
    oner = np.ones((1, 128), dtype=np.float32)

    in_maps = []
    for c in range(NCORES):
        asp = aspect[c * BPC : (c + 1) * BPC]          # [BPC, D]
        aspt = np.ascontiguousarray(
            asp.T.reshape(2, 128, BPC).transpose(1, 0, 2)
        )                                               # [128, 2, BPC]
        in_maps.append(
            {
                "mem": memory[c * BPC : (c + 1) * BPC],
                "wmb": wmb,
                "aspt": aspt,
                "wac": wac,
                "bsc": bsc,
                "onec": onec,
                "oner": oner,
                "winv": winv,
            }
        )
    return in_maps


def run(inputs, trace=False):
    """Returns (out [B, D] float32, exec_time_ns or None)."""
    nc = _get_nc()
    in_maps = make_in_maps(**inputs)
    res = run_bass_kernel_spmd(
        nc, in_maps, core_ids=list(range(NCORES)), trace=trace
    )
    out = np.concatenate(
        [res.results[c]["out"].reshape(BPC, D) for c in range(NCORES)], axis=0
    )
    return out, res.exec_time_ns


def kernel(aspect, memory, W, b):
    out, _ = run(dict(aspect=aspect, memory=memory, W=W, b=b))
    return out


# revision 37
# speedup vs baseline: 1.1760x; 1.1359x over previous
"""AttMemoryLayer Trainium2 kernel (8 NeuronCores, batch-parallel).

Math (per batch b):
    scores[s] = sum_d memory[b,s,d] * W[:D]  (+ c_b, c_b = aspect[b]@W[D:] + b)
    p = exp(tanh(scores))          # tanh in [-1,1] => no max-subtraction needed
    out[b] = (sum_s p[s] * memory[b,s,:]) / sum_s p[s]

Distribution: data-parallel over B=64 across 8 cores (8 batches/core), W/b
replicated, no collectives.  Tiny W/aspect preprocessing (Wm broadcast, Wa
columns, aspect transpose, 1/Wm) is host-side numpy layout prep; all
O(B*S*D) work runs on-device.

Per-core dataflow (per batch: 2 chunks of 16 s-slices):
  - SWDGE cast-DMA streams each 2MB chunk f32->bf16 as [128, 16, 256]
    (partition = s_outer, free = (s_inner, d)); the 32MB f32 HBM read/core
    is the wall-clock floor.
  - One VectorE bf16 2x multiply per chunk: prod = chunk * Wm (broadcast).
  - scores (a free-dim contraction PE cannot do in this layout) split across
    engines: 11 slices tree-reduce on VectorE (4 bf16 halving adds into a
    separate tile + one f32-accumulating reduce_sum over [.,.,16]); 5 slices
    accum-reduce on ScalarE via activation(Copy, accum_out), dump in PSUM.
  - ScalarE: tanh (bias = per-batch c, broadcast via a K=1 ones matmul), exp.
  - PE: 16 accumulating bf16 matmuls per chunk read PROD (not raw memory),
    lhsT = p[:,j] stationary 1-column; the Wm factor is undone at the end by
    one per-batch row multiply with host-precomputed 1/Wm.  Pooling from prod
    keeps memory-tile lifetimes to just the multiply, so DMA slot recycling
    never waits on the exp->matmul chain (this removed both DMA stalls and
    most run-to-run variance).
  - Per-batch normalizer: ones-matmul partition reduce -> reciprocal ->
    ScalarE Copy-with-scale folds the divide into the PSUM->stage copy; one
    8KB output DMA at the end.  Custom slim TileContext tail (one barrier).

Measured on the axon-tunneled TRN2: ~108-110us NEFF exec in quiet fleet
phases (116-129us when the shared tunnel degrades), rel err ~3e-3 vs the
f32 reference (bf16 memory quantization dominates the error).
"""
import sys

for _p in ("/opt/trn_rl_repo",):
    if _p not in sys.path:
        sys.path.append(_p)

import numpy as np

import concourse.bass as bass
import concourse.mybir as mybir
from concourse.tile import TileContext
from concourse.vector_clock import ScopedClock
from concourse.bass_utils import run_bass_kernel_spmd

F32 = mybir.dt.float32
BF16 = mybir.dt.bfloat16

B, S, D = 64, 4096, 256
NCORES = 8
BPC = B // NCORES          # batches per core
SO = 128                   # s_outer (partitions)
SI = S // SO               # s_inner per batch (32)
NCH = 2                    # compute chunks per batch (halves of one DMA)
CJ = SI // NCH             # s-slices per compute chunk (16)


def _split_multi_waits(nc, max_waits=1):
    """This container's walrus build rejects instructions carrying more than
    one sync-wait ("Too many sync wait commands").  Move extra waits onto
    single-wait NoOps inserted immediately before the instruction on the same
    engine; per-engine program order makes this semantics-preserving."""
    cnt = 0
    for bb in nc.main_func.blocks:
        newlist = []
        dirty = False
        for ins in bb.instructions:
            si = ins.sync_info
            if si is not None and si.on_wait and len(si.on_wait) > max_waits:
                waits = list(si.on_wait)
                head, tail = waits[:-max_waits], waits[-max_waits:]
                for w in head:
                    cnt += 1
                    newlist.append(
                        mybir.InstNoOp(
                            name=f"WSPLIT-{cnt}",
                            engine=ins.engine,
                            bass_nofuse=True,
                            sync_info=mybir.SyncInfo(on_wait=[w], on_update=[]),
                        )
                    )
                ins.sync_info = mybir.SyncInfo(
                    on_wait=tail, on_update=list(si.on_update or [])
                )
                dirty = True
            newlist.append(ins)
        if dirty:
            bb.instructions = newlist
    return cnt


class _TC(TileContext):
    """TileContext with a slimmer kernel tail: the drain still waits on all
    outstanding work (output visibility) and semaphores are still cleared
    (repeat-execution safety), but the second all-engine barrier is dropped
    -- NEFF completion already requires every engine stream (including the
    clears) to retire, so nothing can observe a stale semaphore."""

    def _drain_and_barrier(self, tick_clock, wait_clock):
        drain_inst = self.nc.sync.drain()
        wait_clock.add_sem_waits(
            drain_inst.ins, ScopedClock({None: tick_clock.global_clock})
        )
        self.nc.all_engine_barrier()
        popped = self.nc._tile_sem_poison_stack.pop()
        assert popped is self._sem_poison
        self.nc.clear_and_free_semaphores(list(self.sems.allocated().values()))


def build_nc():
    nc = bass.Bass(trn_type="TRN2")

    MEM = nc.dram_tensor("mem", [BPC, S, D], F32, kind="ExternalInput")
    WMB = nc.dram_tensor("wmb", [128, D], F32, kind="ExternalInput")
    ASPT = nc.dram_tensor("aspt", [128, 2, BPC], F32, kind="ExternalInput")
    WAC = nc.dram_tensor("wac", [128, 2], F32, kind="ExternalInput")
    BSC = nc.dram_tensor("bsc", [1, 1], F32, kind="ExternalInput")
    ONEC = nc.dram_tensor("onec", [128, 1], F32, kind="ExternalInput")
    ONER = nc.dram_tensor("oner", [1, 128], F32, kind="ExternalInput")
    WINV = nc.dram_tensor("winv", [1, D], F32, kind="ExternalInput")
    OUT = nc.dram_tensor("out", [1, BPC * D], F32, kind="ExternalOutput")

    mult = mybir.AluOpType.mult
    Act = mybir.ActivationFunctionType

    with _TC(nc) as tc:
        with (
            tc.tile_pool(name="const", bufs=1) as cpool,
            tc.tile_pool(name="mem", bufs=5) as mpool,
            tc.tile_pool(name="small", bufs=8) as spool,
            tc.tile_pool(name="prods", bufs=8) as prpool,
            tc.tile_pool(name="psums", bufs=1, space="PSUM") as pps,
            tc.tile_pool(name="psumm", bufs=2, space="PSUM") as ppm,
        ):
            # ---- constants / setup -------------------------------------
            wmb16 = cpool.tile([128, D], BF16)
            nc.gpsimd.dma_start(wmb16[:], WMB[:])  # f32 -> bf16 cast DMA
            aspt = cpool.tile([128, 2, BPC], F32)
            nc.sync.dma_start(aspt[:], ASPT[:])
            wac = cpool.tile([128, 2], F32)
            nc.sync.dma_start(wac[:], WAC[:])
            bsc = cpool.tile([1, 1], F32)
            nc.sync.dma_start(bsc[:], BSC[:])
            onec = cpool.tile([128, 1], F32)
            nc.sync.dma_start(onec[:], ONEC[:])
            oner = cpool.tile([1, 128], F32)
            nc.sync.dma_start(oner[:], ONER[:])
            winv = cpool.tile([1, D], F32)
            nc.sync.dma_start(winv[:], WINV[:])

            # c_row[1, BPC] = aspect @ Wa + b   (contract d over partitions)
            ps_c = pps.tile([1, BPC], F32)
            nc.tensor.matmul(ps_c[:], lhsT=wac[:, 0:1], rhs=aspt[:, 0, :],
                             start=True, stop=False)
            nc.tensor.matmul(ps_c[:], lhsT=wac[:, 1:2], rhs=aspt[:, 1, :],
                             start=False, stop=True)
            crow = cpool.tile([1, BPC], F32)
            nc.vector.tensor_scalar_add(crow[:], ps_c[:], bsc[0:1, 0:1])

            # c broadcast to all 128 partitions: ones_row.T @ c_row (K=1)
            ps_cb = pps.tile([128, BPC], F32)
            nc.tensor.matmul(ps_cb[:], lhsT=oner[:], rhs=crow[:],
                             start=True, stop=True)
            cb = cpool.tile([128, BPC], F32)
            nc.scalar.copy(cb[:], ps_cb[:])

            MAXCH = 4
            lpart = cpool.tile([128, BPC * MAXCH], F32)  # per-(batch,chunk) l
            stage = cpool.tile([1, BPC, D], F32)         # un-normalized outputs

            # ---- main loop: batches x chunks ---------------------------
            # Uniform two 2MB chunks per batch, 5 ScalarE-reduced slices per
            # chunk: every non-uniform variation of this plan measured worse.
            plans = [[(16, 5)] * 2] * BPC
            for i in range(BPC):
                plan = plans[i]
                nch = len(plan)
                out_ps = ppm.tile([1, D], F32, tag="out_ps")
                s0 = 0
                for c, (cj, nsp) in enumerate(plan):
                    bts = mpool.tile([128, cj, D], BF16, tag="bt")
                    nc.gpsimd.dma_start(
                        bts[:],
                        MEM[i].rearrange("(so si) d -> so si d", so=SO)[
                            :, s0 : s0 + cj, :
                        ],
                    )
                    s0 += cj
                    # scores[s,j] = sum_d bts[s,j,d] * Wm[d]; one bf16 2x
                    # multiply, then reduction split across engines: slices
                    # [0:na) tree-reduce on VectorE, slices [na:cj) accum-
                    # reduce on ScalarE.
                    na = cj - nsp
                    prod = prpool.tile([128, cj, D], BF16, tag="prod")
                    nc.vector.tensor_tensor(
                        prod[:], bts[:],
                        wmb16[:, None, :].to_broadcast((128, cj, D)), mult,
                    )
                    scores = spool.tile([128, cj], F32, tag="scores")
                    dump = ppm.tile([128, D], F32, tag="dump")
                    for j in range(na, cj):
                        nc.scalar.activation(
                            dump[:], prod[:, j, :], Act.Copy,
                            accum_out=scores[:, j : j + 1],
                        )
                    tree = prpool.tile([128, na, 128], BF16, tag="tree")
                    nc.vector.tensor_add(tree[:], prod[:, 0:na, 0:128],
                                         prod[:, 0:na, 128:256])
                    nc.vector.tensor_add(tree[:, :, 0:64], tree[:, :, 0:64],
                                         tree[:, :, 64:128])
                    nc.vector.tensor_add(tree[:, :, 0:32], tree[:, :, 0:32],
                                         tree[:, :, 32:64])
                    nc.vector.tensor_add(tree[:, :, 0:16], tree[:, :, 0:16],
                                         tree[:, :, 16:32])
                    nc.vector.reduce_sum(scores[:, 0:na], tree[:, :, 0:16],
                                         axis=mybir.AxisListType.X)

                    th = spool.tile([128, cj], F32, tag="th")
                    nc.scalar.activation(th[:], scores[:], Act.Tanh,
                                         bias=cb[:, i : i + 1])
                    p16 = spool.tile([128, cj], BF16, tag="p16")
                    nc.scalar.activation(p16[:], th[:], Act.Exp)
                    nc.vector.reduce_sum(
                        lpart[:, i * MAXCH + c : i * MAXCH + c + 1], p16[:],
                        axis=mybir.AxisListType.X,
                    )

                    for j in range(cj):
                        nc.tensor.matmul(
                            out_ps[:], lhsT=p16[:, j : j + 1],
                            rhs=prod[:, j, :],
                            start=(c == 0 and j == 0),
                            stop=(c == nch - 1 and j == cj - 1),
                        )
                # per-batch normalizer: partition-reduce l, reciprocal,
                # and scale the PSUM row on its way into the staging tile
                ps_l = ppm.tile([1, MAXCH], F32, tag="ps_l")
                nc.tensor.matmul(ps_l[:, 0:nch], lhsT=onec[:],
                                 rhs=lpart[:, i * MAXCH : i * MAXCH + nch],
                                 start=True, stop=True)
                lsum = spool.tile([1, 1], F32, tag="lsum")
                nc.vector.reduce_sum(lsum[:], ps_l[:, 0:nch],
                                     axis=mybir.AxisListType.X)
                lrec = spool.tile([1, 1], F32, tag="lrec")
                nc.vector.reciprocal(lrec[:], lsum[:])
                nc.scalar.activation(stage[0:1, i, :], out_ps[:], Act.Copy,
                                     bias=0.0, scale=lrec[0:1, 0:1])
                nc.gpsimd.tensor_tensor(stage[0:1, i, :], stage[0:1, i, :],
                                        winv[0:1, :], mult)

            nc.sync.dma_start(OUT[:], stage[:].rearrange("p i j -> p (i j)"))

    _split_multi_waits(nc)
    return nc


_NC_CACHE = None


def _get_nc():
    global _NC_CACHE
    if _NC_CACHE is None:
        _NC_CACHE = build_nc()
    return _NC_CACHE


def make_in_maps(aspect, memory, W, b):
    aspect = np.asarray(aspect, dtype=np.float32).reshape(B, D)
    memory = np.ascontiguousarray(np.asarray(memory, dtype=np.float32))
    W = np.asarray(W, dtype=np.float32).reshape(2 * D)
    b = np.asarray(b, dtype=np.float32).reshape(1)

    wmb = np.ascontiguousarray(np.tile(W[:D][None, :], (128, 1)))
    wm16 = wmb[0].astype(np.dtype("uint16").newbyteorder("="), copy=False)  # placeholder
    wmq = W[:D].astype(np.float32)
    # quantize Wm exactly as the cast-DMA does (bf16 round-to-nearest-even)
    import ml_dtypes
    wmq16 = wmq.astype(ml_dtypes.bfloat16).astype(np.float32)
    winv = np.where(wmq16 == 0.0, 0.0, 1.0 / wmq16).astype(np.float32)[None, :]
    wac = np.ascontiguousarray(W[D:].reshape(2, 128).T)
    bsc = b.reshape(1, 1)
    onec = np.ones((128, 1), dtype=np.float32)
    oner = np.ones((1, 128), dtype=np.float32)

    in_maps = []
    for c in range(NCORES):
        asp = aspect[c * BPC : (c + 1) * BPC]          # [BPC, D]
        aspt = np.ascontiguousarray(
            asp.T.reshape(2, 128, BPC).transpose(1, 0, 2)
        )                                               # [128, 2, BPC]
        in_maps.append(
            {
                "mem": memory[c * BPC : (c + 1) * BPC],
                "wmb": wmb,
                "aspt": aspt,
                "wac": wac,
                "bsc": bsc,
                "onec": onec,
                "oner": oner,
                "winv": winv,
            }
        )
    return in_maps


def run(inputs, trace=False):
    """Returns (out [B, D] float32, exec_time_ns or None)."""
    nc = _get_nc()
    in_maps = make_in_maps(**inputs)
    res = run_bass_kernel_spmd(
        nc, in_maps, core_ids=list(range(NCORES)), trace=trace
    )
    out = np.concatenate(
        [res.results[c]["out"].reshape(BPC, D) for c in range(NCORES)], axis=0
    )
    return out, res.exec_time_ns


def kernel(aspect, memory, W, b):
    out, _ = run(dict(aspect=aspect, memory=memory, W=W, b=b))
    return out
